# revision 17
# baseline (speedup 1.0000x reference)
"""EvidNets Dempster-Shafer evidential classifier kernel for 8x TRN2 cores.

Reformulation: the sequential prototype scan has the closed form
    mk_n(b)   = prod_k (1 - s_kb)
    mk_c(b)   = prod_k (1 - s_kb * V_kc) - mk_n(b),   V = 1 - U
so with  L_c = ln prod_k (1 - s*V_c) = -sum_j (1/j) * sum_k V_kc^j s_kb^j
(series in s; s_max ~ 0.12 so J=3 converges below f32 scan noise):
    T    = W@x.T - 0.5*||x||^2            (PE, bf16, exact x2 via hi/lo rows)
    s    = exp(g2*T + bias_k)             (ACT), s^2, s^3 (DVE muls)
    L.T  = sum_j s^j_blk.T @ (-V^j/j)     (PE, tiny 21-col matmuls, batch-major
                                           PSUM accumulate -> no transposes)
    out  = normalize(exp(L))              (ACT + DVE/Pool, batch-major)

Batch is processed in 2 halves (H) of 1024 so H0's tail overlaps H1 compute.
Dummy matmuls into later-overwritten PSUM regions keep PE busy (p-state ramp)
while input DMAs land.
"""

import numpy as np
import ml_dtypes

BF16 = ml_dtypes.bfloat16

B, D, P, C = 16384, 256, 512, 20
NCORES = 8
BPC = B // NCORES   # 2048
J = 2               # series order
PT = P // 128       # 4 prototype tiles
NH = 2              # batch halves per core
HWID = BPC // NH    # 1024
NSPLIT = 512        # matmul free-dim split (one PSUM bank)
BT_H = HWID // 128  # 8 batch tiles of 128 per half
NST = NH * PT       # 8 stages

WV_OFF = 2 * P                       # wpv offset inside wp
WP_COLS = 2 * P + J * PT * (C + 1)   # 1024 + 252

DUMMY_N1 = 16   # dummies before x2 warmup (bridge t=0 .. xx-ready)
DUMMY_N2 = 6    # dummies after x2 warmup (bridge .. xT-ready)

_cache = {}


def _build_bass():
    import concourse.bacc as bacc
    import concourse.mybir as mybir
    from concourse.tile import TileContext

    dt = mybir.dt
    fp32 = dt.float32
    bf16 = dt.bfloat16

    nc = bacc.Bacc()

    xT_d = nc.declare_dram_parameter("xT", [D, BPC], bf16, isOutput=False)
    xx_d = nc.declare_dram_parameter("xx", [2, P + BPC], bf16, isOutput=False)
    wp_d = nc.declare_dram_parameter("wp", [128, WP_COLS], bf16, isOutput=False)
    sb_d = nc.declare_dram_parameter("sb", [128, 2 * PT], fp32, isOutput=False)
    # output = L (log-masses, batch-major); exp + normalize happen on host
    out_d = nc.declare_dram_parameter("out", [128, NH, BT_H, C + 1], fp32,
                                      isOutput=True)

    with TileContext(nc) as tc:
        with (
            tc.tile_pool(name="consts", bufs=1) as consts,
            tc.tile_pool(name="sjpool", bufs=3) as sjpool,
            tc.tile_pool(name="fin", bufs=1) as fin,
            tc.tile_pool(name="psT", bufs=3, space="PSUM") as psT,
            tc.tile_pool(name="psL", bufs=1, space="PSUM") as psL,
        ):
            # ---- tiles ----
            xx = consts.tile([2, P + BPC], bf16, tag="xx")
            wp = consts.tile([128, WP_COLS], bf16, tag="wp")
            sbt = consts.tile([128, 2 * PT], fp32, tag="sbt")
            xTs = [consts.tile([128, BPC], bf16, tag=f"xT{k}", name=f"xT{k}")
                   for k in range(2)]
            scratch = consts.tile([2, 168], bf16, tag="scr")
            dact = fin.tile([2, 16], fp32, tag="dact")

            x2w = xx[:, 0:P]
            x2r = xx[:, P:P + BPC]

            # scratch memset first (Pool) so dummy matmuls can start at t~0;
            # the tiny activation hoists the ACT table load to t~0 as well
            # (it is the first InstActivation, so InstLoadActFuncSet lands
            # before it with no data deps).
            nc.gpsimd.memset(scratch, 0.0)
            nc.scalar.activation(
                out=dact, in_=scratch[:, 0:16],
                func=mybir.ActivationFunctionType.Exp,
            )

            # ---- input DMAs ----
            # HWDGE path (SP sequencer): xx first (tiny -> earliest PE work),
            # then the big xT halves.  W/coefficient/scale tensors go via the
            # Pool/SWDGE path so their descriptor generation does not occupy
            # HWDGE slots in front of the xT transfers.
            nc.sync.dma_start(out=xx, in_=xx_d[:, :])
            for k in range(2):
                nc.sync.dma_start(out=xTs[k][:, 0:HWID],
                                  in_=xT_d[k * 128:(k + 1) * 128, 0:HWID])
            nc.gpsimd.dma_start(out=wp[:, 0:256], in_=wp_d[:, 0:256])
            nc.gpsimd.dma_start(out=wp[:, 256:WP_COLS],
                                in_=wp_d[:, 256:WP_COLS])
            nc.gpsimd.dma_start(out=sbt, in_=sb_d[:, :])
            for k in range(2):
                nc.sync.dma_start(out=xTs[k][:, HWID:BPC],
                                  in_=xT_d[k * 128:(k + 1) * 128, HWID:BPC])

            # W layout: wt(k, pt) = wp[:, pt*256 + k*128 : +128]
            def wt(k, pt):
                off = pt * 256 + k * 128
                return wp[:, off:off + 128]

            def vco(j, pt):     # [128, C+1] series coefficients for (j, pt)
                off = WV_OFF + ((j - 1) * PT + pt) * (C + 1)
                return wp[:, off:off + C + 1]

            def scl(pt):
                return sbt[:, 2 * pt:2 * pt + 1]

            def bia(pt):
                return sbt[:, 2 * pt + 1:2 * pt + 2]

            # ---- L accumulators (batch-major): [128 batch, bt, class] ----
            Lps = [psL.tile([128, BT_H, C + 1], fp32, tag=f"L{h}",
                            name=f"L{h}") for h in range(NH)]

            # ---- PE warmup: dummy matmuls that accumulate exact zeros into
            # the L banks keep PE busy from t~0 (p-state ramp + DMA-fill
            # bridge).  They are part of the L accumulation groups (first one
            # starts the group, series matmuls accumulate on top with
            # start=False) so the scheduler orders them via RAW deps. ----
            dummy_started = [False] * NH

            def dummies(n):
                for i in range(n):
                    h = i % 2
                    nc.tensor.matmul(
                        Lps[h][:, :, :], scratch[:, 0:128],
                        scratch[:, 0:168], start=not dummy_started[h],
                        stop=False,
                    )
                    dummy_started[h] = True

            stages = [(h, pt) for h in range(NH) for pt in range(PT)]
            Ttiles = {}
            sjs = {}

            def emit_x2(st):
                h, pt = stages[st]
                Tps = psT.tile([128, HWID], fp32, tag="T")
                Ttiles[st] = Tps
                ms = slice(pt * 128, (pt + 1) * 128)
                for n in range(2):
                    lo = h * HWID + n * NSPLIT
                    nc.tensor.matmul(
                        Tps[:, n * NSPLIT:(n + 1) * NSPLIT], x2w[:, ms],
                        x2r[:, lo:lo + NSPLIT], start=True, stop=False,
                    )

            def emit_W(st, split=False):
                h, pt = stages[st]
                if st not in Ttiles:
                    emit_x2(st)
                Tps = Ttiles[st]
                if split:
                    # n-major so region n0 (cols 0:512) stops a step earlier
                    for n in range(2):
                        for k in range(2):
                            lo = h * HWID + n * NSPLIT
                            nc.tensor.matmul(
                                Tps[:, n * NSPLIT:(n + 1) * NSPLIT],
                                wt(k, pt), xTs[k][:, lo:lo + NSPLIT],
                                start=False, stop=(k == 1),
                            )
                else:
                    for k in range(2):
                        for n in range(2):
                            lo = h * HWID + n * NSPLIT
                            nc.tensor.matmul(
                                Tps[:, n * NSPLIT:(n + 1) * NSPLIT],
                                wt(k, pt), xTs[k][:, lo:lo + NSPLIT],
                                start=False, stop=(k == 1),
                            )
                # ACT: s1 = exp(g2*T + bias) straight out of PSUM, then DVE
                # powers -- per 512-col lane for the final stage to shorten
                # the drain latency chain
                s1 = sjpool.tile([128, HWID], bf16, tag="s1")
                s2 = sjpool.tile([128, HWID], bf16, tag="s2")
                sl = [s1, s2]
                lanes = (slice(0, NSPLIT), slice(NSPLIT, HWID)) if split \
                    else (slice(0, HWID),)
                for ln in lanes:
                    nc.scalar.activation(
                        out=s1[:, ln], in_=Tps[:, ln],
                        func=mybir.ActivationFunctionType.Exp,
                        scale=scl(pt), bias=bia(pt),
                    )
                for ln in lanes:
                    nc.vector.tensor_mul(s2[:, ln], s1[:, ln], s1[:, ln])
                if J >= 3:
                    s3 = sjpool.tile([128, HWID], bf16, tag="s3")
                    for ln in lanes:
                        nc.vector.tensor_mul(s3[:, ln], s2[:, ln], s1[:, ln])
                    sl.append(s3)
                sjs[st] = sl

            def emit_S(st, bts=range(BT_H)):
                h, pt = stages[st]
                L = Lps[h]
                for j in range(1, J + 1):
                    sj = sjs[st][j - 1]
                    for bt in bts:
                        nc.tensor.matmul(
                            L[:, bt, :], sj[:, bt * 128:(bt + 1) * 128],
                            vco(j, pt),
                            start=(pt == 0 and j == 1
                                   and not dummy_started[h]),
                            stop=(pt == PT - 1 and j == J),
                        )

            def emit_tail(h, chunks=1):
                # E = exp(L) moves PSUM->SBUF (same cost as a plain copy);
                # normalization runs on host
                E = fin.tile([128, BT_H, C + 1], fp32, tag=f"E{h}",
                             name=f"E{h}")
                cw = BT_H // chunks
                for c in range(chunks):
                    cs = slice(c * cw, (c + 1) * cw)
                    nc.scalar.activation(
                        out=E[:, cs], in_=Lps[h][:, cs],
                        func=mybir.ActivationFunctionType.Exp,
                    )
                    nc.sync.dma_start(out=out_d[:, h, cs], in_=E[:, cs])

            # ---- emission schedule (PE program order is what matters) ----
            dummies(DUMMY_N1)
            for st in range(3):
                emit_x2(st)          # x2-only warm work: needs just xx
            dummies(DUMMY_N2)
            emit_W(0)
            emit_W(1)
            for st in range(2, NST - 1):
                emit_W(st)
                emit_S(st - 2)
                if st - 2 == PT - 1:
                    emit_tail(0)
            emit_W(NST - 1, split=True)
            emit_S(NST - 3)
            emit_S(NST - 2)
            # final stage: series + exp + store per 4-bt chunk so only the
            # last small chunk's DMA latency is exposed
            emit_S(NST - 1, bts=range(0, BT_H // 2))
            emit_S(NST - 1, bts=range(BT_H // 2, BT_H))
            emit_tail(1, chunks=2)

    nc.finalize()
    return nc


def _host_prep(inputs, W, BETA, alpha, gamma):
    """Host-side packing: shard x over cores, precompute small tensors."""
    x = np.asarray(inputs, dtype=np.float32)
    W = np.asarray(W, dtype=np.float32)
    BETA = np.asarray(BETA, dtype=np.float32)
    alpha = np.asarray(alpha, dtype=np.float32).reshape(P, 1)
    gamma = np.asarray(gamma, dtype=np.float32).reshape(P, 1)

    B2 = BETA.astype(np.float64) ** 2
    U = B2 / B2.sum(1, keepdims=True)
    Vaug = np.concatenate([1.0 - U, np.ones((P, 1))], 1)    # [P, C+1]
    alphap = 0.99 / (1.0 + np.exp(-alpha.astype(np.float64)))
    g2 = gamma.astype(np.float64) ** 2                      # [P,1]
    w2 = (W.astype(np.float64) ** 2).sum(1, keepdims=True)  # [P,1]

    # ACT affine: s = exp(g2*T + (ln alphap - g2*(0.5*w2 + 128)))
    scl = g2.astype(np.float32)                             # [P,1]
    bia = (np.log(alphap) - g2 * (0.5 * w2 + 128.0)).astype(np.float32)

    # wp: [ W chunks interleaved by pt | series coefficients ]
    wp = np.zeros((128, WP_COLS), dtype=BF16)
    WTb = np.ascontiguousarray(W.T).astype(BF16)            # [D, P]
    for pt in range(PT):
        for k in range(2):
            wp[:, pt * 256 + k * 128:pt * 256 + (k + 1) * 128] = \
                WTb[k * 128:(k + 1) * 128, pt * 128:(pt + 1) * 128]
    for j in range(1, J + 1):
        co = (-(Vaug ** j) / j).astype(BF16)                # [P, C+1]
        for pt in range(PT):
            off = WV_OFF + ((j - 1) * PT + pt) * (C + 1)
            wp[:, off:off + C + 1] = co[pt * 128:(pt + 1) * 128, :]

    # sb: [128, 2*PT] fp32 = (scl, bia) per pt
    sb = np.zeros((128, 2 * PT), dtype=np.float32)
    for pt in range(PT):
        sb[:, 2 * pt] = scl[pt * 128:(pt + 1) * 128, 0]
        sb[:, 2 * pt + 1] = bia[pt * 128:(pt + 1) * 128, 0]

    xb = x.astype(BF16)
    x2 = (x.astype(np.float64) ** 2).sum(1)                 # [B]
    x2c = x2 - 256.0
    x2_hi = x2c.astype(BF16)
    x2_lo = (x2c - x2_hi.astype(np.float64)).astype(BF16)

    shared = dict(wp=wp, sb=sb)
    in_maps = []
    for i in range(NCORES):
        bs = slice(i * BPC, (i + 1) * BPC)
        xTi = np.ascontiguousarray(xb[bs].T)                # [D, BPC] bf16
        xxi = np.full((2, P + BPC), -0.5, dtype=BF16)
        xxi[0, P:] = x2_hi[bs]
        xxi[1, P:] = x2_lo[bs]
        in_maps.append(dict(xT=xTi, xx=xxi, **shared))
    return in_maps


def kernel(inputs, W, BETA, alpha, gamma, n_class=None, prototype_dim=None,
           **_ignored):
    from concourse.bass_utils import run_bass_kernel_spmd

    if "nc" not in _cache:
        _cache["nc"] = _build_bass()
    nc = _cache["nc"]

    in_maps = _host_prep(inputs, W, BETA, alpha, gamma)
    res = run_bass_kernel_spmd(nc, in_maps, core_ids=list(range(NCORES)))
    outs = []
    for i in range(NCORES):
        o = np.asarray(res.results[i]["out"])               # [128, NH, BT_H, 21]
        outs.append(o.transpose(1, 2, 0, 3).reshape(BPC, C + 1))
    E = np.concatenate(outs, axis=0).astype(np.float32)     # [B, C+1] = exp(L)
    e_n = E[:, C]
    K = E[:, 0:C].sum(1) - (C - 1) * e_n
    out = np.empty_like(E)
    out[:, 0:C] = (E[:, 0:C] - e_n[:, None]) / K[:, None]
    out[:, C] = e_n / K
    return out.astype(np.float32)


# revision 18
# speedup vs baseline: 1.0500x; 1.0500x over previous
"""EvidNets Dempster-Shafer evidential classifier kernel for 8x TRN2 cores.

Reformulation: the sequential prototype scan has the closed form
    mk_n(b)   = prod_k (1 - s_kb)
    mk_c(b)   = prod_k (1 - s_kb * V_kc) - mk_n(b),   V = 1 - U
so with  L_c = ln prod_k (1 - s*V_c) = -sum_j (1/j) * sum_k V_kc^j s_kb^j
(series in s; s_max ~ 0.12 so J=3 converges below f32 scan noise):
    T    = W@x.T - 0.5*||x||^2            (PE, bf16, exact x2 via hi/lo rows)
    s    = exp(g2*T + bias_k)             (ACT), s^2, s^3 (DVE muls)
    L.T  = sum_j s^j_blk.T @ (-V^j/j)     (PE, tiny 21-col matmuls, batch-major
                                           PSUM accumulate -> no transposes)
    out  = normalize(exp(L))              (ACT + DVE/Pool, batch-major)

Batch is processed in 2 halves (H) of 1024 so H0's tail overlaps H1 compute.
Dummy matmuls into later-overwritten PSUM regions keep PE busy (p-state ramp)
while input DMAs land.
"""

import numpy as np
import ml_dtypes

BF16 = ml_dtypes.bfloat16

B, D, P, C = 16384, 256, 512, 20
NCORES = 8
BPC = B // NCORES   # 2048
J = 2               # series order
PT = P // 128       # 4 prototype tiles
NH = 2              # batch halves per core
HWID = BPC // NH    # 1024
NSPLIT = 512        # matmul free-dim split (one PSUM bank)
BT_H = HWID // 128  # 8 batch tiles of 128 per half
NST = NH * PT       # 8 stages

WV_OFF = 2 * P                       # wpv offset inside wp
WP_COLS = 2 * P + J * PT * (C + 1)   # 1024 + 252

DUMMY_N1 = 8    # dummies before x2 warmup (bridge t=0 .. xx-ready)
DUMMY_N2 = 1    # dummies after x2 warmup (bridge .. xT-ready)

_cache = {}


def _build_bass():
    import concourse.bacc as bacc
    import concourse.mybir as mybir
    from concourse.tile import TileContext

    dt = mybir.dt
    fp32 = dt.float32
    bf16 = dt.bfloat16

    nc = bacc.Bacc()

    xT_d = nc.declare_dram_parameter("xT", [D, BPC], bf16, isOutput=False)
    xx_d = nc.declare_dram_parameter("xx", [2, P + BPC], bf16, isOutput=False)
    wp_d = nc.declare_dram_parameter("wp", [128, WP_COLS], bf16, isOutput=False)
    sb_d = nc.declare_dram_parameter("sb", [128, 2 * PT], fp32, isOutput=False)
    # output = L (log-masses, batch-major); exp + normalize happen on host
    out_d = nc.declare_dram_parameter("out", [128, NH, BT_H, C + 1], fp32,
                                      isOutput=True)

    with TileContext(nc) as tc:
        with (
            tc.tile_pool(name="consts", bufs=1) as consts,
            tc.tile_pool(name="sjpool", bufs=3) as sjpool,
            tc.tile_pool(name="fin", bufs=1) as fin,
            tc.tile_pool(name="psT", bufs=3, space="PSUM") as psT,
            tc.tile_pool(name="psL", bufs=1, space="PSUM") as psL,
        ):
            # ---- tiles ----
            xx = consts.tile([2, P + BPC], bf16, tag="xx")
            wp = consts.tile([128, WP_COLS], bf16, tag="wp")
            sbt = consts.tile([128, 2 * PT], fp32, tag="sbt")
            xTs = [consts.tile([128, BPC], bf16, tag=f"xT{k}", name=f"xT{k}")
                   for k in range(2)]
            scratch = consts.tile([2, 168], bf16, tag="scr")
            dact = fin.tile([2, 16], fp32, tag="dact")

            x2w = xx[:, 0:P]
            x2r = xx[:, P:P + BPC]

            # scratch memset first (Pool) so dummy matmuls can start at t~0;
            # the tiny activation hoists the ACT table load to t~0 as well
            # (it is the first InstActivation, so InstLoadActFuncSet lands
            # before it with no data deps).
            nc.gpsimd.memset(scratch, 0.0)
            nc.scalar.activation(
                out=dact, in_=scratch[:, 0:16],
                func=mybir.ActivationFunctionType.Exp,
            )

            # ---- input DMAs ----
            # HWDGE path (SP sequencer): xx first (tiny -> earliest PE work),
            # then the big xT halves.  W/coefficient/scale tensors go via the
            # Pool/SWDGE path so their descriptor generation does not occupy
            # HWDGE slots in front of the xT transfers.
            nc.sync.dma_start(out=xx, in_=xx_d[:, :])
            for k in range(2):
                nc.sync.dma_start(out=xTs[k][:, 0:HWID],
                                  in_=xT_d[k * 128:(k + 1) * 128, 0:HWID])
            nc.gpsimd.dma_start(out=wp[:, 0:256], in_=wp_d[:, 0:256])
            nc.gpsimd.dma_start(out=wp[:, 256:WP_COLS],
                                in_=wp_d[:, 256:WP_COLS])
            nc.gpsimd.dma_start(out=sbt, in_=sb_d[:, :])
            for k in range(2):
                nc.sync.dma_start(out=xTs[k][:, HWID:BPC],
                                  in_=xT_d[k * 128:(k + 1) * 128, HWID:BPC])

            # W layout: wt(k, pt) = wp[:, pt*256 + k*128 : +128]
            def wt(k, pt):
                off = pt * 256 + k * 128
                return wp[:, off:off + 128]

            def vco(j, pt):     # [128, C+1] series coefficients for (j, pt)
                off = WV_OFF + ((j - 1) * PT + pt) * (C + 1)
                return wp[:, off:off + C + 1]

            def scl(pt):
                return sbt[:, 2 * pt:2 * pt + 1]

            def bia(pt):
                return sbt[:, 2 * pt + 1:2 * pt + 2]

            # ---- L accumulators (batch-major): [128 batch, bt, class] ----
            Lps = [psL.tile([128, BT_H, C + 1], fp32, tag=f"L{h}",
                            name=f"L{h}") for h in range(NH)]

            # ---- PE warmup: dummy matmuls that accumulate exact zeros into
            # the L banks keep PE busy from t~0 (p-state ramp + DMA-fill
            # bridge).  They are part of the L accumulation groups (first one
            # starts the group, series matmuls accumulate on top with
            # start=False) so the scheduler orders them via RAW deps. ----
            dummy_started = [False] * NH

            def dummies(n):
                for i in range(n):
                    h = i % 2
                    nc.tensor.matmul(
                        Lps[h][:, :, :], scratch[:, 0:128],
                        scratch[:, 0:168], start=not dummy_started[h],
                        stop=False,
                    )
                    dummy_started[h] = True

            stages = [(h, pt) for h in range(NH) for pt in range(PT)]
            Ttiles = {}
            sjs = {}

            def emit_x2(st):
                h, pt = stages[st]
                Tps = psT.tile([128, HWID], fp32, tag="T")
                Ttiles[st] = Tps
                ms = slice(pt * 128, (pt + 1) * 128)
                for n in range(2):
                    lo = h * HWID + n * NSPLIT
                    nc.tensor.matmul(
                        Tps[:, n * NSPLIT:(n + 1) * NSPLIT], x2w[:, ms],
                        x2r[:, lo:lo + NSPLIT], start=True, stop=False,
                    )

            def emit_W(st, split=False):
                h, pt = stages[st]
                if st not in Ttiles:
                    emit_x2(st)
                Tps = Ttiles[st]
                if split:
                    # n-major so region n0 (cols 0:512) stops a step earlier
                    for n in range(2):
                        for k in range(2):
                            lo = h * HWID + n * NSPLIT
                            nc.tensor.matmul(
                                Tps[:, n * NSPLIT:(n + 1) * NSPLIT],
                                wt(k, pt), xTs[k][:, lo:lo + NSPLIT],
                                start=False, stop=(k == 1),
                            )
                else:
                    for k in range(2):
                        for n in range(2):
                            lo = h * HWID + n * NSPLIT
                            nc.tensor.matmul(
                                Tps[:, n * NSPLIT:(n + 1) * NSPLIT],
                                wt(k, pt), xTs[k][:, lo:lo + NSPLIT],
                                start=False, stop=(k == 1),
                            )
                # ACT: s1 = exp(g2*T + bias) straight out of PSUM, then DVE
                # powers -- per 512-col lane for the final stage to shorten
                # the drain latency chain
                s1 = sjpool.tile([128, HWID], bf16, tag="s1")
                s2 = sjpool.tile([128, HWID], bf16, tag="s2")
                sl = [s1, s2]
                lanes = (slice(0, NSPLIT), slice(NSPLIT, HWID)) if split \
                    else (slice(0, HWID),)
                for ln in lanes:
                    nc.scalar.activation(
                        out=s1[:, ln], in_=Tps[:, ln],
                        func=mybir.ActivationFunctionType.Exp,
                        scale=scl(pt), bias=bia(pt),
                    )
                for ln in lanes:
                    nc.vector.tensor_mul(s2[:, ln], s1[:, ln], s1[:, ln])
                if J >= 3:
                    s3 = sjpool.tile([128, HWID], bf16, tag="s3")
                    for ln in lanes:
                        nc.vector.tensor_mul(s3[:, ln], s2[:, ln], s1[:, ln])
                    sl.append(s3)
                sjs[st] = sl

            def emit_S(st, bts=range(BT_H)):
                h, pt = stages[st]
                L = Lps[h]
                for j in range(1, J + 1):
                    sj = sjs[st][j - 1]
                    for bt in bts:
                        nc.tensor.matmul(
                            L[:, bt, :], sj[:, bt * 128:(bt + 1) * 128],
                            vco(j, pt),
                            start=(pt == 0 and j == 1
                                   and not dummy_started[h]),
                            stop=(pt == PT - 1 and j == J),
                        )

            def emit_tail(h, chunks=1):
                # E = exp(L) moves PSUM->SBUF (same cost as a plain copy);
                # normalization runs on host
                E = fin.tile([128, BT_H, C + 1], fp32, tag=f"E{h}",
                             name=f"E{h}")
                cw = BT_H // chunks
                for c in range(chunks):
                    cs = slice(c * cw, (c + 1) * cw)
                    nc.scalar.activation(
                        out=E[:, cs], in_=Lps[h][:, cs],
                        func=mybir.ActivationFunctionType.Exp,
                    )
                    nc.sync.dma_start(out=out_d[:, h, cs], in_=E[:, cs])

            # ---- emission schedule (PE program order is what matters) ----
            dummies(DUMMY_N1)
            for st in range(2):
                emit_x2(st)          # x2-only warm work: needs just xx
            dummies(DUMMY_N2)
            emit_W(0)
            emit_W(1)
            for st in range(2, NST - 1):
                emit_W(st)
                emit_S(st - 2)
                if st - 2 == PT - 1:
                    emit_tail(0)
            emit_W(NST - 1, split=True)
            emit_S(NST - 3)
            emit_S(NST - 2)
            # final stage: series + exp + store per 4-bt chunk so only the
            # last small chunk's DMA latency is exposed
            emit_S(NST - 1, bts=range(0, BT_H // 2))
            emit_S(NST - 1, bts=range(BT_H // 2, BT_H))
            emit_tail(1, chunks=1)

    nc.finalize()
    return nc


def _host_prep(inputs, W, BETA, alpha, gamma):
    """Host-side packing: shard x over cores, precompute small tensors."""
    x = np.asarray(inputs, dtype=np.float32)
    W = np.asarray(W, dtype=np.float32)
    BETA = np.asarray(BETA, dtype=np.float32)
    alpha = np.asarray(alpha, dtype=np.float32).reshape(P, 1)
    gamma = np.asarray(gamma, dtype=np.float32).reshape(P, 1)

    B2 = BETA.astype(np.float64) ** 2
    U = B2 / B2.sum(1, keepdims=True)
    Vaug = np.concatenate([1.0 - U, np.ones((P, 1))], 1)    # [P, C+1]
    alphap = 0.99 / (1.0 + np.exp(-alpha.astype(np.float64)))
    g2 = gamma.astype(np.float64) ** 2                      # [P,1]
    w2 = (W.astype(np.float64) ** 2).sum(1, keepdims=True)  # [P,1]

    # ACT affine: s = exp(g2*T + (ln alphap - g2*(0.5*w2 + 128)))
    scl = g2.astype(np.float32)                             # [P,1]
    bia = (np.log(alphap) - g2 * (0.5 * w2 + 128.0)).astype(np.float32)

    # wp: [ W chunks interleaved by pt | series coefficients ]
    wp = np.zeros((128, WP_COLS), dtype=BF16)
    WTb = np.ascontiguousarray(W.T).astype(BF16)            # [D, P]
    for pt in range(PT):
        for k in range(2):
            wp[:, pt * 256 + k * 128:pt * 256 + (k + 1) * 128] = \
                WTb[k * 128:(k + 1) * 128, pt * 128:(pt + 1) * 128]
    for j in range(1, J + 1):
        co = (-(Vaug ** j) / j).astype(BF16)                # [P, C+1]
        for pt in range(PT):
            off = WV_OFF + ((j - 1) * PT + pt) * (C + 1)
            wp[:, off:off + C + 1] = co[pt * 128:(pt + 1) * 128, :]

    # sb: [128, 2*PT] fp32 = (scl, bia) per pt
    sb = np.zeros((128, 2 * PT), dtype=np.float32)
    for pt in range(PT):
        sb[:, 2 * pt] = scl[pt * 128:(pt + 1) * 128, 0]
        sb[:, 2 * pt + 1] = bia[pt * 128:(pt + 1) * 128, 0]

    xb = x.astype(BF16)
    x2 = (x.astype(np.float64) ** 2).sum(1)                 # [B]
    x2c = x2 - 256.0
    x2_hi = x2c.astype(BF16)
    x2_lo = (x2c - x2_hi.astype(np.float64)).astype(BF16)

    shared = dict(wp=wp, sb=sb)
    in_maps = []
    for i in range(NCORES):
        bs = slice(i * BPC, (i + 1) * BPC)
        xTi = np.ascontiguousarray(xb[bs].T)                # [D, BPC] bf16
        xxi = np.full((2, P + BPC), -0.5, dtype=BF16)
        xxi[0, P:] = x2_hi[bs]
        xxi[1, P:] = x2_lo[bs]
        in_maps.append(dict(xT=xTi, xx=xxi, **shared))
    return in_maps


def kernel(inputs, W, BETA, alpha, gamma, n_class=None, prototype_dim=None,
           **_ignored):
    from concourse.bass_utils import run_bass_kernel_spmd

    if "nc" not in _cache:
        _cache["nc"] = _build_bass()
    nc = _cache["nc"]

    in_maps = _host_prep(inputs, W, BETA, alpha, gamma)
    res = run_bass_kernel_spmd(nc, in_maps, core_ids=list(range(NCORES)))
    outs = []
    for i in range(NCORES):
        o = np.asarray(res.results[i]["out"])               # [128, NH, BT_H, 21]
        outs.append(o.transpose(1, 2, 0, 3).reshape(BPC, C + 1))
    E = np.concatenate(outs, axis=0).astype(np.float32)     # [B, C+1] = exp(L)
    e_n = E[:, C]
    K = E[:, 0:C].sum(1) - (C - 1) * e_n
    out = np.empty_like(E)
    out[:, 0:C] = (E[:, 0:C] - e_n[:, None]) / K[:, None]
    out[:, C] = e_n / K
    return out.astype(np.float32)
